# revision 39
# baseline (speedup 1.0000x reference)
"""Trainium2 Bass kernel for 16-head MHA (B=2, L=2048, D=1024), 8 NeuronCores.

Sharding: 8 cores = 4 head-groups x 2 batches. Core c handles head group
hg = c // 2 (4 heads = 256 of the 1024 projection columns) for batch
b = c % 2. Per core, for its batch:
  - qhT/khT/vhT slices (256, 2048) [head-dim on partitions, seq free],
    fp16 operands, fp32 PSUM accumulation.
  - vhT is DMA-transposed into an augmented V layout: per key tile, 4
    head blocks of [64 dims | ones column], so the P@V matmul (M=65
    stationary) also produces the softmax row sums.
  - attention in 8 rounds of (head-pair, query-quarter): S_T scores
    (keys on partitions, 2-head row-tiling), one wide exp on ScalarE
    (softmax scale folded into the activation's affine), P@V
    accumulation over key tiles, then reciprocal + K=1-matmul broadcast
    + multiply normalization off the critical path (double-buffered
    PSUM).
  - row-packed output projection against Wo -> partial (2048, 1024).
Host sums the 4 head-group partials per batch and adds bo.
"""

import sys

sys.path.insert(0, "/opt/trn_rl_repo")

import numpy as np

import concourse.bass as bass  # noqa: F401  (registers types)
import concourse.mybir as mybir
import concourse.tile as tile
from concourse import bacc
from concourse.bass import ds, ts
from concourse.bass_utils import run_bass_kernel_spmd

F32 = mybir.dt.float32
F16 = mybir.dt.float16
AF = mybir.ActivationFunctionType

D = 1024          # model dim
L = 2048          # sequence length
B = 2             # batch
NH = 16           # total heads
HD = 64           # head dim
HS = 256          # head-slice columns per core (4 heads)
HC = HD + 1       # head block width in the augmented V layout
KT = D // 128     # 8 contraction tiles for projections
LT = L // 128     # 16 key tiles
N_CORES = 8

_PROGRAM = None


def _build_program():
    nc = bacc.Bacc(
        "TRN2",
        target_bir_lowering=False,
        debug=False,
        enable_asserts=False,
        num_devices=N_CORES,
    )
    xqT = nc.dram_tensor("xqT", (D, L), F16, kind="ExternalInput").ap()
    xkT = nc.dram_tensor("xkT", (D, L), F16, kind="ExternalInput").ap()
    xvT = nc.dram_tensor("xvT", (D, L), F16, kind="ExternalInput").ap()
    wqT = nc.dram_tensor("wqT", (D, HS), F16, kind="ExternalInput").ap()
    wkT = nc.dram_tensor("wkT", (D, HS), F16, kind="ExternalInput").ap()
    wvT = nc.dram_tensor("wvT", (D, HS), F16, kind="ExternalInput").ap()
    woT = nc.dram_tensor("woT", (HS, D), F16, kind="ExternalInput").ap()
    bqkv = nc.dram_tensor("bqkv", (128, 6), F32, kind="ExternalInput").ap()
    onesv = nc.dram_tensor("onesv", (128, LT, 4), F16, kind="ExternalInput").ap()
    onesr = nc.dram_tensor("onesr", (1, 64), F16, kind="ExternalInput").ap()
    out = nc.dram_tensor("out", (L, D), F32, kind="ExternalOutput").ap()

    with tile.TileContext(nc) as tc:
        _emit(nc, tc, xqT, xkT, xvT, wqT, wkT, wvT, woT, bqkv, onesv, onesr, out)
    nc.compile()
    return nc


def _emit(nc, tc, xqT, xkT, xvT, wqT, wkT, wvT, woT, bqkv, onesv, onesr, out):
    with (
        tc.tile_pool(name="const", bufs=1) as constp,
        tc.tile_pool(name="wpool", bufs=1) as wpool,
        tc.tile_pool(name="proj", bufs=1) as projp,
        tc.tile_pool(name="xt", bufs=6) as xtp,
        tc.tile_pool(name="pt", bufs=4) as ptp,
        tc.tile_pool(name="small", bufs=4) as smallp,
        tc.tile_pool(name="outsb", bufs=3) as outp,
    ):
        # --- constants ---
        bqkv_sb = constp.tile([128, 6], F32)
        nc.sync.dma_start(bqkv_sb[:], bqkv)
        onesr_sb = constp.tile([1, 64], F16)
        nc.sync.dma_start(onesr_sb[:], onesr)

        # --- persistent activations ---
        qh_sb = [projp.tile([128, L], F16, tag=f"qh{m}", name=f"qh{m}") for m in range(2)]
        kh_sb = [projp.tile([128, L], F16, tag=f"kh{m}", name=f"kh{m}") for m in range(2)]
        vt_sb = [projp.tile([128, L], F16, tag=f"vt{m}", name=f"vt{m}") for m in range(2)]
        # augmented V: per key tile, 4 head blocks of [64 dims | ones col]
        vh_sb = projp.tile([128, LT, 4 * HC], F16, tag="vh", name="vh")
        on_sb = [projp.tile([128, L], F16, tag=f"on{p}", name=f"on{p}") for p in range(2)]
        vh4 = vh_sb[:].rearrange("p t (h c) -> p t h c", c=HC)

        def load_w(name, src):
            t = wpool.tile([128, KT, HS], F16, tag=name, name=name)
            nc.sync.dma_start(t[:], src.rearrange("(t p) c -> p t c", p=128))
            return t

        # --- phase 1: q/k/v projections (head-dim on partitions) ---
        # weight loads are emitted just-in-time so the xq stream + q-proj
        # start as early as possible
        with tc.tile_pool(name="pjps", bufs=2, space="PSUM") as pA:
            for which, (xdram, wname, wsrc, dst, bias0) in enumerate((
                (xqT, "wq", wqT, qh_sb, 0),
                (xkT, "wk", wkT, kh_sb, 2),
                (xvT, "wv", wvT, vt_sb, 4),
            )):
                w_sb = load_w(wname, wsrc)
                ps = [pA.tile([128, L], F32, tag="pj", name="pjps") for _ in range(2)]
                for t in range(KT):
                    xt_ = xtp.tile([128, L], F16, tag="xt")
                    for nn in range(4):
                        nc.sync.dma_start(
                            xt_[:, ts(nn, 512)], xdram[ts(t, 128), ts(nn, 512)]
                        )
                    for m in range(2):
                        for n in range(4):
                            nc.tensor.matmul(
                                ps[m][:, ts(n, 512)],
                                lhsT=w_sb[:, t, ts(m, 128)],
                                rhs=xt_[:, ts(n, 512)],
                                start=(t == 0),
                                stop=(t == KT - 1),
                            )
                for m in range(2):
                    nc.vector.tensor_scalar_add(
                        dst[m][:], ps[m][:], bqkv_sb[:, ds(bias0 + m, 1)]
                    )

        # late weights/constants (not needed until attention/out-proj)
        wo_sb = []
        for p in range(2):
            t = wpool.tile([128, D], F16, tag=f"wo{p}", name=f"wo{p}")
            nc.sync.dma_start(t[:], woT[ts(p, 128), :])
            wo_sb.append(t)
        # ones columns of the augmented V layout (col 64 of each head block)
        nc.sync.dma_start(vh4[:, :, :, ds(HD, 1)], onesv.unsqueeze(-1))

        # transpose vhT -> augmented vh layout (keys on partitions)
        for p in range(2):
            for m in range(LT):
                stg = smallp.tile([128, 128], F16, tag="stg", name="stg")
                nc.sync.dma_start(stg[:], vt_sb[p][:, ts(m, 128)], transpose=True)
                nc.vector.tensor_copy(
                    vh4[:, m, ds(2 * p, 2), ds(0, HD)],
                    stg[:].rearrange("p (h c) -> p h c", c=HD),
                )

        # --- phase 2: attention, 8 rounds of (query-quarter, head-pair) ---
        with tc.tile_pool(name="atps", bufs=2, space="PSUM") as pC:

            def normalize(p, qq, o_ps):
                # row 64 of each O bank holds the exp row sums. reciprocal
                # into partition 0 of rb, then a log2 DMA-doubling broadcast
                # across partitions (SBUF-only: leaves the hot PSUM slots
                # alone; latency hides inside the next round).
                for h2 in range(2):
                    ot = o_ps[h2]
                    rb = smallp.tile([64, 512], F16, tag="rb", name="rb")
                    with nc.allow_low_precision(reason="softmax recip"):
                        nc.vector.reciprocal(rb[ds(0, 1), :], ot[ds(HD, 1), :])
                    # two-hop parallel broadcast: 1 -> 8 -> 64 partitions
                    for j in range(1, 8):
                        nc.sync.dma_start(rb[ds(j, 1), :], rb[ds(0, 1), :])
                    for j in range(1, 8):
                        nc.sync.dma_start(rb[ds(8 * j, 8), :], rb[ds(0, 8), :])
                    dst_sl = ts(qq, 512)
                    if h2 == 0:
                        nc.vector.tensor_mul(
                            on_sb[p][ds(0, HD), dst_sl],
                            ot[ds(0, HD), :],
                            rb[:],
                        )
                    else:
                        om = smallp.tile([64, 512], F16, tag="om", name="om")
                        nc.vector.tensor_mul(om[:], ot[ds(0, HD), :], rb[:])
                        # partition shift 0-63 -> 64-127 via DMA
                        nc.sync.dma_start(on_sb[p][ds(64, HD), dst_sl], om[:])

            def emit_scores_exp(p, qq, t):
                s_ps = pC.tile([128, 1024], F32, tag="s", name="s_ps")
                for h2 in range(2):
                    nc.tensor.matmul(
                        s_ps[:, ts(h2, 512)],
                        lhsT=kh_sb[p][ds(h2 * 64, 64), ts(t, 128)],
                        rhs=qh_sb[p][ds(h2 * 64, 64), ts(qq, 512)],
                        start=True,
                        stop=True,
                        tile_position=(h2 * 64, 0),
                    )
                p_t = ptp.tile([128, 1024], F16, tag="pt", name="p_t")
                nc.scalar.activation(p_t[:], s_ps[:], AF.Exp, scale=0.125)
                return p_t

            def emit_pv(p, o_ps, p_t, t):
                for h2 in range(2):
                    nc.tensor.matmul(
                        o_ps[h2][ds(0, HC), :],
                        lhsT=vh_sb[:, t, ds((2 * p + h2) * HC, HC)],
                        rhs=p_t[:, ts(h2, 512)],
                        start=(t == 0),
                        stop=(t == LT - 1),
                    )

            # software-pipelined emission: P@V for key tile t is emitted
            # after scores/exp for t+1, so the PE never stalls waiting for
            # the exp it just requested. Normalization of the previous round
            # and the output projection of the previous quarter are emitted
            # mid-round so their latency hides behind the streaming loop.
            pending_norm = None
            for qq in range(4):
                for p in range(2):
                    o_ps = [
                        pC.tile([128, 512], F32, tag=f"o{h2}", name=f"o{h2}")
                        for h2 in range(2)
                    ]
                    prev = emit_scores_exp(p, qq, 0)
                    for t in range(1, LT):
                        p_t = emit_scores_exp(p, qq, t)
                        emit_pv(p, o_ps, prev, t - 1)
                        prev = p_t
                        if t == 2 and pending_norm is not None:
                            normalize(*pending_norm)
                            pending_norm = None
                    emit_pv(p, o_ps, prev, LT - 1)
                    pending_norm = (p, qq, o_ps)
            normalize(*pending_norm)

        # --- phase 3: output projection tail ---
        # Each head-pair's two heads are stacked on partitions 0-127 of
        # on_sb[p] / wo_sb[p], so one K=128 matmul per pair contracts over
        # both heads at once. Quarters 0-2 first so the final quarter's
        # norm latency hides behind them.
        with tc.tile_pool(name="opps", bufs=4, space="PSUM") as pD:
            for qt in list(range(12)) + list(range(12, LT)):
                out_t = outp.tile([128, D], F32, tag="ot", name="out_t")
                for oc in range(2):
                    psA = pD.tile([128, 512], F32, tag="opA", name="psA")
                    for p in range(2):
                        nc.tensor.matmul(
                            psA[:],
                            lhsT=on_sb[p][:, ts(qt, 128)],
                            rhs=wo_sb[p][:, ts(oc, 512)],
                            start=(p == 0),
                            stop=(p == 1),
                        )
                    nc.vector.tensor_copy(out_t[:, ts(oc, 512)], psA[:])
                nc.sync.dma_start(out[ts(qt, 128), :], out_t[:])


def get_program():
    global _PROGRAM
    if _PROGRAM is None:
        _PROGRAM = _build_program()
    return _PROGRAM


def prepare_in_maps(q, k, v, Wq, bq, Wk, bk, Wv, bv, Wo, bo):
    """Build the 8 per-core input dicts (host-side slicing/transposes)."""
    q = np.asarray(q, dtype=np.float32)
    k = np.asarray(k, dtype=np.float32)
    v = np.asarray(v, dtype=np.float32)
    xT = {}
    for b in range(B):
        xT[("q", b)] = np.ascontiguousarray(q[b].T).astype(np.float16)
        xT[("k", b)] = np.ascontiguousarray(k[b].T).astype(np.float16)
        xT[("v", b)] = np.ascontiguousarray(v[b].T).astype(np.float16)
    ones_v = np.ones((128, LT, 4), dtype=np.float16)
    ones_r = np.ones((1, 64), dtype=np.float16)
    in_maps = []
    for c in range(N_CORES):
        hg, b = c // 2, c % 2
        hs = hg * HS
        bq_s = np.asarray(bq, np.float32)[hs : hs + HS]
        bk_s = np.asarray(bk, np.float32)[hs : hs + HS]
        bv_s = np.asarray(bv, np.float32)[hs : hs + HS]
        bqkv_m = np.stack(
            [
                bq_s[0:128],
                bq_s[128:256],
                bk_s[0:128],
                bk_s[128:256],
                bv_s[0:128],
                bv_s[128:256],
            ],
            axis=1,
        )
        in_maps.append(
            {
                "xqT": xT[("q", b)],
                "xkT": xT[("k", b)],
                "xvT": xT[("v", b)],
                "wqT": np.asarray(Wq, np.float32)[hs : hs + HS, :].T.astype(np.float16),
                "wkT": np.asarray(Wk, np.float32)[hs : hs + HS, :].T.astype(np.float16),
                "wvT": np.asarray(Wv, np.float32)[hs : hs + HS, :].T.astype(np.float16),
                "woT": np.asarray(Wo, np.float32)[:, hs : hs + HS].T.astype(np.float16),
                "bqkv": np.ascontiguousarray(bqkv_m),
                "onesv": ones_v,
                "onesr": ones_r,
            }
        )
    return in_maps


def combine_outputs(results, bo):
    """Sum head-group partials per batch and add the output bias."""
    bo = np.asarray(bo, np.float32)
    full = np.zeros((B, L, D), dtype=np.float32)
    for c in range(N_CORES):
        hg, b = c // 2, c % 2
        full[b] += results[c]["out"]
    full += bo
    return full


def run(inputs, trace=False, trace_cores=None):
    nc = get_program()
    in_maps = prepare_in_maps(**inputs)
    res = run_bass_kernel_spmd(
        nc,
        in_maps,
        core_ids=list(range(N_CORES)),
        trace=trace,
        trace_cores=trace_cores,
    )
    out = combine_outputs(res.results, inputs["bo"])
    return out, res


def kernel(**inputs):
    out, _ = run(inputs, trace=False)
    return out


# revision 40
# speedup vs baseline: 1.0707x; 1.0707x over previous
"""Trainium2 Bass kernel for 16-head MHA (B=2, L=2048, D=1024), 8 NeuronCores.

Sharding: 8 cores = 4 head-groups x 2 batches. Core c handles head group
hg = c // 2 (4 heads = 256 of the 1024 projection columns) for batch
b = c % 2. Per core, for its batch:
  - qhT/khT/vhT slices (256, 2048) [head-dim on partitions, seq free],
    fp16 operands, fp32 PSUM accumulation.
  - vhT is DMA-transposed into an augmented V layout: per key tile, 4
    head blocks of [64 dims | ones column], so the P@V matmul (M=65
    stationary) also produces the softmax row sums.
  - attention in 8 rounds of (head-pair, query-quarter): S_T scores
    (keys on partitions, 2-head row-tiling), one wide exp on ScalarE
    (softmax scale folded into the activation's affine), P@V
    accumulation over key tiles, then reciprocal + K=1-matmul broadcast
    + multiply normalization off the critical path (double-buffered
    PSUM).
  - row-packed output projection against Wo -> partial (2048, 1024).
Host sums the 4 head-group partials per batch and adds bo.
"""

import sys

sys.path.insert(0, "/opt/trn_rl_repo")

import numpy as np

import concourse.bass as bass  # noqa: F401  (registers types)
import concourse.mybir as mybir
import concourse.tile as tile
from concourse import bacc
from concourse.bass import ds, ts
from concourse.bass_utils import run_bass_kernel_spmd

F32 = mybir.dt.float32
F16 = mybir.dt.float16
AF = mybir.ActivationFunctionType

D = 1024          # model dim
L = 2048          # sequence length
B = 2             # batch
NH = 16           # total heads
HD = 64           # head dim
HS = 256          # head-slice columns per core (4 heads)
HC = HD + 1       # head block width in the augmented V layout
KT = D // 128     # 8 contraction tiles for projections
LT = L // 128     # 16 key tiles
N_CORES = 8

_PROGRAM = None


def _build_program():
    nc = bacc.Bacc(
        "TRN2",
        target_bir_lowering=False,
        debug=False,
        enable_asserts=False,
        num_devices=N_CORES,
    )
    xqT = nc.dram_tensor("xqT", (D, L), F16, kind="ExternalInput").ap()
    xkT = nc.dram_tensor("xkT", (D, L), F16, kind="ExternalInput").ap()
    xvT = nc.dram_tensor("xvT", (D, L), F16, kind="ExternalInput").ap()
    wqT = nc.dram_tensor("wqT", (D, HS), F16, kind="ExternalInput").ap()
    wkT = nc.dram_tensor("wkT", (D, HS), F16, kind="ExternalInput").ap()
    wvT = nc.dram_tensor("wvT", (D, HS), F16, kind="ExternalInput").ap()
    woT = nc.dram_tensor("woT", (HS, D), F16, kind="ExternalInput").ap()
    bqkv = nc.dram_tensor("bqkv", (128, 6), F32, kind="ExternalInput").ap()
    onesv = nc.dram_tensor("onesv", (128, LT, 4), F16, kind="ExternalInput").ap()
    onesr = nc.dram_tensor("onesr", (1, 64), F16, kind="ExternalInput").ap()
    out = nc.dram_tensor("out", (L, D), F32, kind="ExternalOutput").ap()

    with tile.TileContext(nc) as tc:
        _emit(nc, tc, xqT, xkT, xvT, wqT, wkT, wvT, woT, bqkv, onesv, onesr, out)
    nc.compile()
    return nc


def _emit(nc, tc, xqT, xkT, xvT, wqT, wkT, wvT, woT, bqkv, onesv, onesr, out):
    with (
        tc.tile_pool(name="const", bufs=1) as constp,
        tc.tile_pool(name="wpool", bufs=1) as wpool,
        tc.tile_pool(name="proj", bufs=1) as projp,
        tc.tile_pool(name="xt", bufs=6) as xtp,
        tc.tile_pool(name="pt", bufs=4) as ptp,
        tc.tile_pool(name="small", bufs=4) as smallp,
        tc.tile_pool(name="outsb", bufs=3) as outp,
    ):
        # --- constants ---
        bqkv_sb = constp.tile([128, 6], F32)
        nc.sync.dma_start(bqkv_sb[:], bqkv)
        onesr_sb = constp.tile([1, 64], F16)
        nc.sync.dma_start(onesr_sb[:], onesr)

        # --- persistent activations ---
        qh_sb = [projp.tile([128, L], F16, tag=f"qh{m}", name=f"qh{m}") for m in range(2)]
        kh_sb = [projp.tile([128, L], F16, tag=f"kh{m}", name=f"kh{m}") for m in range(2)]
        vt_sb = [projp.tile([128, L], F16, tag=f"vt{m}", name=f"vt{m}") for m in range(2)]
        # augmented V: per key tile, 4 head blocks of [64 dims | ones col]
        vh_sb = projp.tile([128, LT, 4 * HC], F16, tag="vh", name="vh")
        on_sb = [projp.tile([128, L], F16, tag=f"on{p}", name=f"on{p}") for p in range(2)]
        vh4 = vh_sb[:].rearrange("p t (h c) -> p t h c", c=HC)

        def load_w(name, src):
            t = wpool.tile([128, KT, HS], F16, tag=name, name=name)
            nc.sync.dma_start(t[:], src.rearrange("(t p) c -> p t c", p=128))
            return t

        # --- phase 1: q/k/v projections (head-dim on partitions) ---
        # weight loads are emitted just-in-time so the xq stream + q-proj
        # start as early as possible
        with tc.tile_pool(name="pjps", bufs=2, space="PSUM") as pA:
            for which, (xdram, wname, wsrc, dst, bias0) in enumerate((
                (xqT, "wq", wqT, qh_sb, 0),
                (xkT, "wk", wkT, kh_sb, 2),
                (xvT, "wv", wvT, vt_sb, 4),
            )):
                w_sb = load_w(wname, wsrc)
                ps = [pA.tile([128, L], F32, tag="pj", name="pjps") for _ in range(2)]
                for t in range(KT):
                    xt_ = xtp.tile([128, L], F16, tag="xt")
                    nc.sync.dma_start(xt_[:, ds(0, 1024)], xdram[ts(t, 128), ds(0, 1024)])
                    nc.sync.dma_start(xt_[:, ds(1024, 1024)], xdram[ts(t, 128), ds(1024, 1024)])
                    for m in range(2):
                        for n in range(4):
                            nc.tensor.matmul(
                                ps[m][:, ts(n, 512)],
                                lhsT=w_sb[:, t, ts(m, 128)],
                                rhs=xt_[:, ts(n, 512)],
                                start=(t == 0),
                                stop=(t == KT - 1),
                            )
                for m in range(2):
                    nc.vector.tensor_scalar_add(
                        dst[m][:], ps[m][:], bqkv_sb[:, ds(bias0 + m, 1)]
                    )

        # late weights/constants (not needed until attention/out-proj)
        wo_sb = []
        for p in range(2):
            t = wpool.tile([128, D], F16, tag=f"wo{p}", name=f"wo{p}")
            nc.sync.dma_start(t[:], woT[ts(p, 128), :])
            wo_sb.append(t)
        # ones columns of the augmented V layout (col 64 of each head block)
        nc.sync.dma_start(vh4[:, :, :, ds(HD, 1)], onesv.unsqueeze(-1))

        # transpose vhT -> augmented vh layout (keys on partitions)
        for p in range(2):
            for m in range(LT):
                stg = smallp.tile([128, 128], F16, tag="stg", name="stg")
                nc.sync.dma_start(stg[:], vt_sb[p][:, ts(m, 128)], transpose=True)
                nc.vector.tensor_copy(
                    vh4[:, m, ds(2 * p, 2), ds(0, HD)],
                    stg[:].rearrange("p (h c) -> p h c", c=HD),
                )

        # --- phase 2: attention, 8 rounds of (query-quarter, head-pair) ---
        with tc.tile_pool(name="atps", bufs=2, space="PSUM") as pC:

            def normalize(p, qq, o_ps):
                # row 64 of each O bank holds the exp row sums. reciprocal
                # into partition 0 of rb, then a log2 DMA-doubling broadcast
                # across partitions (SBUF-only: leaves the hot PSUM slots
                # alone; latency hides inside the next round).
                for h2 in range(2):
                    ot = o_ps[h2]
                    rb = smallp.tile([64, 512], F16, tag="rb", name="rb")
                    with nc.allow_low_precision(reason="softmax recip"):
                        nc.vector.reciprocal(rb[ds(0, 1), :], ot[ds(HD, 1), :])
                    # two-hop parallel broadcast: 1 -> 8 -> 64 partitions
                    for j in range(1, 8):
                        nc.sync.dma_start(rb[ds(j, 1), :], rb[ds(0, 1), :])
                    for j in range(1, 8):
                        nc.sync.dma_start(rb[ds(8 * j, 8), :], rb[ds(0, 8), :])
                    dst_sl = ts(qq, 512)
                    if h2 == 0:
                        nc.vector.tensor_mul(
                            on_sb[p][ds(0, HD), dst_sl],
                            ot[ds(0, HD), :],
                            rb[:],
                        )
                    else:
                        om = smallp.tile([64, 512], F16, tag="om", name="om")
                        nc.vector.tensor_mul(om[:], ot[ds(0, HD), :], rb[:])
                        # partition shift 0-63 -> 64-127 via DMA
                        nc.sync.dma_start(on_sb[p][ds(64, HD), dst_sl], om[:])

            def emit_scores_exp(p, qq, t):
                s_ps = pC.tile([128, 1024], F32, tag="s", name="s_ps")
                for h2 in range(2):
                    nc.tensor.matmul(
                        s_ps[:, ts(h2, 512)],
                        lhsT=kh_sb[p][ds(h2 * 64, 64), ts(t, 128)],
                        rhs=qh_sb[p][ds(h2 * 64, 64), ts(qq, 512)],
                        start=True,
                        stop=True,
                        tile_position=(h2 * 64, 0),
                    )
                p_t = ptp.tile([128, 1024], F16, tag="pt", name="p_t")
                nc.scalar.activation(p_t[:], s_ps[:], AF.Exp, scale=0.125)
                return p_t

            def emit_pv(p, o_ps, p_t, t):
                for h2 in range(2):
                    nc.tensor.matmul(
                        o_ps[h2][ds(0, HC), :],
                        lhsT=vh_sb[:, t, ds((2 * p + h2) * HC, HC)],
                        rhs=p_t[:, ts(h2, 512)],
                        start=(t == 0),
                        stop=(t == LT - 1),
                    )

            # software-pipelined emission: P@V for key tile t is emitted
            # after scores/exp for t+1, so the PE never stalls waiting for
            # the exp it just requested. Normalization of the previous round
            # and the output projection of the previous quarter are emitted
            # mid-round so their latency hides behind the streaming loop.
            pending_norm = None
            for qq in range(4):
                for p in range(2):
                    o_ps = [
                        pC.tile([128, 512], F32, tag=f"o{h2}", name=f"o{h2}")
                        for h2 in range(2)
                    ]
                    prev = emit_scores_exp(p, qq, 0)
                    for t in range(1, LT):
                        p_t = emit_scores_exp(p, qq, t)
                        emit_pv(p, o_ps, prev, t - 1)
                        prev = p_t
                        if t == 2 and pending_norm is not None:
                            normalize(*pending_norm)
                            pending_norm = None
                    emit_pv(p, o_ps, prev, LT - 1)
                    pending_norm = (p, qq, o_ps)
            normalize(*pending_norm)

        # --- phase 3: output projection tail ---
        # Each head-pair's two heads are stacked on partitions 0-127 of
        # on_sb[p] / wo_sb[p], so one K=128 matmul per pair contracts over
        # both heads at once. Quarters 0-2 first so the final quarter's
        # norm latency hides behind them.
        with tc.tile_pool(name="opps", bufs=4, space="PSUM") as pD:
            for qt in list(range(12)) + list(range(12, LT)):
                out_t = outp.tile([128, D], F32, tag="ot", name="out_t")
                for oc in range(2):
                    psA = pD.tile([128, 512], F32, tag="opA", name="psA")
                    for p in range(2):
                        nc.tensor.matmul(
                            psA[:],
                            lhsT=on_sb[p][:, ts(qt, 128)],
                            rhs=wo_sb[p][:, ts(oc, 512)],
                            start=(p == 0),
                            stop=(p == 1),
                        )
                    nc.vector.tensor_copy(out_t[:, ts(oc, 512)], psA[:])
                nc.sync.dma_start(out[ts(qt, 128), :], out_t[:])


def get_program():
    global _PROGRAM
    if _PROGRAM is None:
        _PROGRAM = _build_program()
    return _PROGRAM


def prepare_in_maps(q, k, v, Wq, bq, Wk, bk, Wv, bv, Wo, bo):
    """Build the 8 per-core input dicts (host-side slicing/transposes)."""
    q = np.asarray(q, dtype=np.float32)
    k = np.asarray(k, dtype=np.float32)
    v = np.asarray(v, dtype=np.float32)
    xT = {}
    for b in range(B):
        xT[("q", b)] = np.ascontiguousarray(q[b].T).astype(np.float16)
        xT[("k", b)] = np.ascontiguousarray(k[b].T).astype(np.float16)
        xT[("v", b)] = np.ascontiguousarray(v[b].T).astype(np.float16)
    ones_v = np.ones((128, LT, 4), dtype=np.float16)
    ones_r = np.ones((1, 64), dtype=np.float16)
    in_maps = []
    for c in range(N_CORES):
        hg, b = c // 2, c % 2
        hs = hg * HS
        bq_s = np.asarray(bq, np.float32)[hs : hs + HS]
        bk_s = np.asarray(bk, np.float32)[hs : hs + HS]
        bv_s = np.asarray(bv, np.float32)[hs : hs + HS]
        bqkv_m = np.stack(
            [
                bq_s[0:128],
                bq_s[128:256],
                bk_s[0:128],
                bk_s[128:256],
                bv_s[0:128],
                bv_s[128:256],
            ],
            axis=1,
        )
        in_maps.append(
            {
                "xqT": xT[("q", b)],
                "xkT": xT[("k", b)],
                "xvT": xT[("v", b)],
                "wqT": np.asarray(Wq, np.float32)[hs : hs + HS, :].T.astype(np.float16),
                "wkT": np.asarray(Wk, np.float32)[hs : hs + HS, :].T.astype(np.float16),
                "wvT": np.asarray(Wv, np.float32)[hs : hs + HS, :].T.astype(np.float16),
                "woT": np.asarray(Wo, np.float32)[:, hs : hs + HS].T.astype(np.float16),
                "bqkv": np.ascontiguousarray(bqkv_m),
                "onesv": ones_v,
                "onesr": ones_r,
            }
        )
    return in_maps


def combine_outputs(results, bo):
    """Sum head-group partials per batch and add the output bias."""
    bo = np.asarray(bo, np.float32)
    full = np.zeros((B, L, D), dtype=np.float32)
    for c in range(N_CORES):
        hg, b = c // 2, c % 2
        full[b] += results[c]["out"]
    full += bo
    return full


def run(inputs, trace=False, trace_cores=None):
    nc = get_program()
    in_maps = prepare_in_maps(**inputs)
    res = run_bass_kernel_spmd(
        nc,
        in_maps,
        core_ids=list(range(N_CORES)),
        trace=trace,
        trace_cores=trace_cores,
    )
    out = combine_outputs(res.results, inputs["bo"])
    return out, res


def kernel(**inputs):
    out, _ = run(inputs, trace=False)
    return out


# revision 41
# speedup vs baseline: 1.2369x; 1.1552x over previous
"""Trainium2 Bass kernel for 16-head MHA (B=2, L=2048, D=1024), 8 NeuronCores.

Sharding: 8 cores = 4 head-groups x 2 batches. Core c handles head group
hg = c // 2 (4 heads = 256 of the 1024 projection columns) for batch
b = c % 2. Per core, for its batch:
  - qhT/khT/vhT slices (256, 2048) [head-dim on partitions, seq free],
    fp16 operands, fp32 PSUM accumulation.
  - vhT is DMA-transposed into an augmented V layout: per key tile, 4
    head blocks of [64 dims | ones column], so the P@V matmul (M=65
    stationary) also produces the softmax row sums.
  - attention in 8 rounds of (head-pair, query-quarter): S_T scores
    (keys on partitions, 2-head row-tiling), one wide exp on ScalarE
    (softmax scale folded into the activation's affine), P@V
    accumulation over key tiles, then reciprocal + K=1-matmul broadcast
    + multiply normalization off the critical path (double-buffered
    PSUM).
  - row-packed output projection against Wo -> partial (2048, 1024).
Host sums the 4 head-group partials per batch and adds bo.
"""

import sys

sys.path.insert(0, "/opt/trn_rl_repo")

import numpy as np

import concourse.bass as bass  # noqa: F401  (registers types)
import concourse.mybir as mybir
import concourse.tile as tile
from concourse import bacc
from concourse.bass import ds, ts
from concourse.bass_utils import run_bass_kernel_spmd

F32 = mybir.dt.float32
F16 = mybir.dt.float16
AF = mybir.ActivationFunctionType

D = 1024          # model dim
L = 2048          # sequence length
B = 2             # batch
NH = 16           # total heads
HD = 64           # head dim
HS = 256          # head-slice columns per core (4 heads)
HC = HD + 1       # head block width in the augmented V layout
KT = D // 128     # 8 contraction tiles for projections
LT = L // 128     # 16 key tiles
N_CORES = 8

_PROGRAM = None


def _build_program():
    nc = bacc.Bacc(
        "TRN2",
        target_bir_lowering=False,
        debug=False,
        enable_asserts=False,
        num_devices=N_CORES,
    )
    xqT = nc.dram_tensor("xqT", (D, L), F16, kind="ExternalInput").ap()
    xkT = nc.dram_tensor("xkT", (D, L), F16, kind="ExternalInput").ap()
    xvT = nc.dram_tensor("xvT", (D, L), F16, kind="ExternalInput").ap()
    wqT = nc.dram_tensor("wqT", (D, HS), F16, kind="ExternalInput").ap()
    wkT = nc.dram_tensor("wkT", (D, HS), F16, kind="ExternalInput").ap()
    wvT = nc.dram_tensor("wvT", (D, HS), F16, kind="ExternalInput").ap()
    woT = nc.dram_tensor("woT", (HS, D), F16, kind="ExternalInput").ap()
    bqkv = nc.dram_tensor("bqkv", (128, 6), F32, kind="ExternalInput").ap()
    onesv = nc.dram_tensor("onesv", (128, LT, 4), F16, kind="ExternalInput").ap()
    onesr = nc.dram_tensor("onesr", (1, 64), F16, kind="ExternalInput").ap()
    out = nc.dram_tensor("out", (L, D), F32, kind="ExternalOutput").ap()

    with tile.TileContext(nc) as tc:
        _emit(nc, tc, xqT, xkT, xvT, wqT, wkT, wvT, woT, bqkv, onesv, onesr, out)
    nc.compile()
    return nc


def _emit(nc, tc, xqT, xkT, xvT, wqT, wkT, wvT, woT, bqkv, onesv, onesr, out):
    with (
        tc.tile_pool(name="const", bufs=1) as constp,
        tc.tile_pool(name="wpool", bufs=1) as wpool,
        tc.tile_pool(name="proj", bufs=1) as projp,
        tc.tile_pool(name="xt", bufs=6) as xtp,
        tc.tile_pool(name="pt", bufs=4) as ptp,
        tc.tile_pool(name="small", bufs=4) as smallp,
        tc.tile_pool(name="outsb", bufs=3) as outp,
    ):
        # --- constants ---
        bqkv_sb = constp.tile([128, 6], F32)
        nc.sync.dma_start(bqkv_sb[:], bqkv)
        onesr_sb = constp.tile([1, 64], F16)
        nc.sync.dma_start(onesr_sb[:], onesr)

        # --- persistent activations ---
        qh_sb = [projp.tile([128, L], F16, tag=f"qh{m}", name=f"qh{m}") for m in range(2)]
        kh_sb = [projp.tile([128, L], F16, tag=f"kh{m}", name=f"kh{m}") for m in range(2)]
        vt_sb = [projp.tile([128, L], F16, tag=f"vt{m}", name=f"vt{m}") for m in range(2)]
        # augmented V: per key tile, 4 head blocks of [64 dims | ones col]
        vh_sb = projp.tile([128, LT, 4 * HC], F16, tag="vh", name="vh")
        on_sb = [projp.tile([128, L], F16, tag=f"on{p}", name=f"on{p}") for p in range(2)]
        vh4 = vh_sb[:].rearrange("p t (h c) -> p t h c", c=HC)

        def load_w(name, src):
            t = wpool.tile([128, KT, HS], F16, tag=name, name=name)
            nc.sync.dma_start(t[:], src.rearrange("(t p) c -> p t c", p=128))
            return t

        # --- phase 1: q/k/v projections (head-dim on partitions) ---
        # weight loads are emitted just-in-time so the xq stream + q-proj
        # start as early as possible
        with tc.tile_pool(name="pjps", bufs=2, space="PSUM") as pA:
            for which, (xdram, wname, wsrc, dst, bias0) in enumerate((
                (xqT, "wq", wqT, qh_sb, 0),
                (xkT, "wk", wkT, kh_sb, 2),
                (xvT, "wv", wvT, vt_sb, 4),
            )):
                w_sb = load_w(wname, wsrc)
                ps = [pA.tile([128, L], F32, tag="pj", name="pjps") for _ in range(2)]
                for t in range(KT):
                    xt_ = xtp.tile([128, L], F16, tag="xt")
                    nc.sync.dma_start(xt_[:, ds(0, 1024)], xdram[ts(t, 128), ds(0, 1024)])
                    nc.sync.dma_start(xt_[:, ds(1024, 1024)], xdram[ts(t, 128), ds(1024, 1024)])
                    for m in range(2):
                        for n in range(4):
                            nc.tensor.matmul(
                                ps[m][:, ts(n, 512)],
                                lhsT=w_sb[:, t, ts(m, 128)],
                                rhs=xt_[:, ts(n, 512)],
                                start=(t == 0),
                                stop=(t == KT - 1),
                            )
                for m in range(2):
                    nc.vector.tensor_scalar_add(
                        dst[m][:], ps[m][:], bqkv_sb[:, ds(bias0 + m, 1)]
                    )

        # late weights/constants (not needed until attention/out-proj)
        wo_sb = []
        for p in range(2):
            t = wpool.tile([128, D], F16, tag=f"wo{p}", name=f"wo{p}")
            nc.sync.dma_start(t[:], woT[ts(p, 128), :])
            wo_sb.append(t)
        # ones columns of the augmented V layout (col 64 of each head block)
        nc.sync.dma_start(vh4[:, :, :, ds(HD, 1)], onesv.unsqueeze(-1))

        # transpose vhT -> augmented vh layout (keys on partitions)
        for p in range(2):
            for m in range(LT):
                stg = smallp.tile([128, 128], F16, tag="stg", name="stg")
                nc.sync.dma_start(stg[:], vt_sb[p][:, ts(m, 128)], transpose=True)
                nc.vector.tensor_copy(
                    vh4[:, m, ds(2 * p, 2), ds(0, HD)],
                    stg[:].rearrange("p (h c) -> p h c", c=HD),
                )

        # --- phase 2: attention, 8 rounds of (query-quarter, head-pair) ---
        with tc.tile_pool(name="atps", bufs=2, space="PSUM") as pC:

            def normalize(p, qq, o_ps):
                # row 64 of each O bank holds the exp row sums. reciprocal
                # into partition 0 of rb, then a log2 DMA-doubling broadcast
                # across partitions (SBUF-only: leaves the hot PSUM slots
                # alone; latency hides inside the next round).
                for h2 in range(2):
                    ot = o_ps[h2]
                    rb = smallp.tile([64, 512], F16, tag="rb", name="rb")
                    with nc.allow_low_precision(reason="softmax recip"):
                        nc.vector.reciprocal(rb[ds(0, 1), :], ot[ds(HD, 1), :])
                    w = 1
                    while w < 64:
                        nc.sync.dma_start(rb[ds(w, w), :], rb[ds(0, w), :])
                        w *= 2
                    dst_sl = ts(qq, 512)
                    if h2 == 0:
                        nc.vector.tensor_mul(
                            on_sb[p][ds(0, HD), dst_sl],
                            ot[ds(0, HD), :],
                            rb[:],
                        )
                    else:
                        om = smallp.tile([64, 512], F16, tag="om", name="om")
                        nc.vector.tensor_mul(om[:], ot[ds(0, HD), :], rb[:])
                        # partition shift 0-63 -> 64-127 via DMA
                        nc.sync.dma_start(on_sb[p][ds(64, HD), dst_sl], om[:])

            def emit_scores_exp(p, qq, t):
                s_ps = pC.tile([128, 1024], F32, tag="s", name="s_ps")
                for h2 in range(2):
                    nc.tensor.matmul(
                        s_ps[:, ts(h2, 512)],
                        lhsT=kh_sb[p][ds(h2 * 64, 64), ts(t, 128)],
                        rhs=qh_sb[p][ds(h2 * 64, 64), ts(qq, 512)],
                        start=True,
                        stop=True,
                        tile_position=(h2 * 64, 0),
                    )
                p_t = ptp.tile([128, 1024], F16, tag="pt", name="p_t")
                nc.scalar.activation(p_t[:], s_ps[:], AF.Exp, scale=0.125)
                return p_t

            def emit_pv(p, o_ps, p_t, t):
                for h2 in range(2):
                    nc.tensor.matmul(
                        o_ps[h2][ds(0, HC), :],
                        lhsT=vh_sb[:, t, ds((2 * p + h2) * HC, HC)],
                        rhs=p_t[:, ts(h2, 512)],
                        start=(t == 0),
                        stop=(t == LT - 1),
                    )

            # software-pipelined emission: P@V for key tile t is emitted
            # after scores/exp for t+1, so the PE never stalls waiting for
            # the exp it just requested. Normalization of the previous round
            # and the output projection of the previous quarter are emitted
            # mid-round so their latency hides behind the streaming loop.
            pending_norm = None
            for qq in range(4):
                for p in range(2):
                    o_ps = [
                        pC.tile([128, 512], F32, tag=f"o{h2}", name=f"o{h2}")
                        for h2 in range(2)
                    ]
                    prev = emit_scores_exp(p, qq, 0)
                    for t in range(1, LT):
                        p_t = emit_scores_exp(p, qq, t)
                        emit_pv(p, o_ps, prev, t - 1)
                        prev = p_t
                        if t == 2 and pending_norm is not None:
                            normalize(*pending_norm)
                            pending_norm = None
                    emit_pv(p, o_ps, prev, LT - 1)
                    pending_norm = (p, qq, o_ps)
            normalize(*pending_norm)

        # --- phase 3: output projection tail ---
        # Each head-pair's two heads are stacked on partitions 0-127 of
        # on_sb[p] / wo_sb[p], so one K=128 matmul per pair contracts over
        # both heads at once. Quarters 0-2 first so the final quarter's
        # norm latency hides behind them.
        with tc.tile_pool(name="opps", bufs=4, space="PSUM") as pD:
            for qt in list(range(12)) + list(range(12, LT)):
                out_t = outp.tile([128, D], F32, tag="ot", name="out_t")
                for oc in range(2):
                    psA = pD.tile([128, 512], F32, tag="opA", name="psA")
                    for p in range(2):
                        nc.tensor.matmul(
                            psA[:],
                            lhsT=on_sb[p][:, ts(qt, 128)],
                            rhs=wo_sb[p][:, ts(oc, 512)],
                            start=(p == 0),
                            stop=(p == 1),
                        )
                    nc.vector.tensor_copy(out_t[:, ts(oc, 512)], psA[:])
                nc.sync.dma_start(out[ts(qt, 128), :], out_t[:])


def get_program():
    global _PROGRAM
    if _PROGRAM is None:
        _PROGRAM = _build_program()
    return _PROGRAM


def prepare_in_maps(q, k, v, Wq, bq, Wk, bk, Wv, bv, Wo, bo):
    """Build the 8 per-core input dicts (host-side slicing/transposes)."""
    q = np.asarray(q, dtype=np.float32)
    k = np.asarray(k, dtype=np.float32)
    v = np.asarray(v, dtype=np.float32)
    xT = {}
    for b in range(B):
        xT[("q", b)] = np.ascontiguousarray(q[b].T).astype(np.float16)
        xT[("k", b)] = np.ascontiguousarray(k[b].T).astype(np.float16)
        xT[("v", b)] = np.ascontiguousarray(v[b].T).astype(np.float16)
    ones_v = np.ones((128, LT, 4), dtype=np.float16)
    ones_r = np.ones((1, 64), dtype=np.float16)
    in_maps = []
    for c in range(N_CORES):
        hg, b = c // 2, c % 2
        hs = hg * HS
        bq_s = np.asarray(bq, np.float32)[hs : hs + HS]
        bk_s = np.asarray(bk, np.float32)[hs : hs + HS]
        bv_s = np.asarray(bv, np.float32)[hs : hs + HS]
        bqkv_m = np.stack(
            [
                bq_s[0:128],
                bq_s[128:256],
                bk_s[0:128],
                bk_s[128:256],
                bv_s[0:128],
                bv_s[128:256],
            ],
            axis=1,
        )
        in_maps.append(
            {
                "xqT": xT[("q", b)],
                "xkT": xT[("k", b)],
                "xvT": xT[("v", b)],
                "wqT": np.asarray(Wq, np.float32)[hs : hs + HS, :].T.astype(np.float16),
                "wkT": np.asarray(Wk, np.float32)[hs : hs + HS, :].T.astype(np.float16),
                "wvT": np.asarray(Wv, np.float32)[hs : hs + HS, :].T.astype(np.float16),
                "woT": np.asarray(Wo, np.float32)[:, hs : hs + HS].T.astype(np.float16),
                "bqkv": np.ascontiguousarray(bqkv_m),
                "onesv": ones_v,
                "onesr": ones_r,
            }
        )
    return in_maps


def combine_outputs(results, bo):
    """Sum head-group partials per batch and add the output bias."""
    bo = np.asarray(bo, np.float32)
    full = np.zeros((B, L, D), dtype=np.float32)
    for c in range(N_CORES):
        hg, b = c // 2, c % 2
        full[b] += results[c]["out"]
    full += bo
    return full


def run(inputs, trace=False, trace_cores=None):
    nc = get_program()
    in_maps = prepare_in_maps(**inputs)
    res = run_bass_kernel_spmd(
        nc,
        in_maps,
        core_ids=list(range(N_CORES)),
        trace=trace,
        trace_cores=trace_cores,
    )
    out = combine_outputs(res.results, inputs["bo"])
    return out, res


def kernel(**inputs):
    out, _ = run(inputs, trace=False)
    return out
